# revision 34
# baseline (speedup 1.0000x reference)
"""Trainium2 Bass kernel for CustomLSTMForecast.

B=512, T=256, I=256, H=512. Data-parallel: batch sharded 8 ways (64
rows/core), LSTM + fc weights replicated.  Measured ~1.37 ms on HW
(vs 1.87 ms for the [64, 512]-layout baseline).

Per-core design (batch m = 64):
  Gate layout: gates land in PSUM with each gate as [128, 256] —
  partitions 0:64 = batch x hidden-lo (0:256), partitions 64:128 =
  batch x hidden-hi (256:512).  Two PSUM tiles per step, FD-packed so
  one sigmoid covers f and i:
     pFI [128, 512]: FD 0:256 = f-gate, 256:512 = i-gate
     pCO [128, 512]: FD 0:256 = c_hat,  256:512 = o-gate
  Every matmul is N=512 over a tile's full FD width (one PSUM
  accumulation chain per tile/partition-half); the host packs W into
  (tile, column-half) quadrants to make those moving slices
  contiguous.  The two column groups (out base partitions 0/64) run
  concurrently on the PE.  K-chunks: 4x h, 2x x, 1x ones row (bias).

  All elementwise runs at [128, 256]/[128, 512] (full partition
  occupancy — FD-bound ACT/DVE cost is nearly halved vs a [64, 512]
  layout).  The h path is bf16 (2x DVE mode); the c path stays f32.

  The serial chain per step is h-matmuls(FI) -> sigmoid(fi) ->
  [tanh(c_hat) -> u2 -> c] -> tanh(c) -> h -> PE transpose -> cast ->
  next h-matmuls.  To shorten it: tanh(c)/h/transpose/cast are
  pipelined in two FD-halves (hidden chunks {0,2} then {1,3}, giving
  the KORD h-matmul order), each hidden chunk's transpose/cast has its
  own tiles and semaphores, x-part matmuls of step t+1 fill the PE
  during the elementwise of step t, and column-group pairs are emitted
  adjacently so the PE co-issues them.

  PE transposes require base-partition-0 operands (base-64 stationary
  inputs hang the exec unit), hence the base-aligned [64, 128] h
  blocks.
"""
from contextlib import ExitStack

import numpy as np

import concourse.bass as bass
import concourse.tile as tile
from concourse import bacc, mybir
from concourse.bass_utils import run_bass_kernel_spmd

F32 = mybir.dt.float32
BF16 = mybir.dt.bfloat16
AF = mybir.ActivationFunctionType

B, T, I, H = 512, 256, 256, 512
NCORES = 8
BC = B // NCORES          # 64 batch rows per core
HH = H // 2               # 256: free size of a [128, 256] gate tile
KH = H // 128             # 4 hidden k-chunks
KX = I // 128             # 2 input k-chunks
NK = KH + KX + 1          # 7 k-chunks incl bias row

# gate chunk order in W_w: f=0, i=1, o=2, chat=3
# pFI holds (f, i) in FD halves; pCO holds (chat, o)
G_FI = (0, 1)
G_CO = (3, 2)
# h-part k-chunk emission order: FD-half a of the elementwise tail
# yields hidden chunks 0 and 2, half b yields 1 and 3
KORD = (0, 2, 1, 3)

_CACHE = {}


def _build(nsteps=T):
    if nsteps in _CACHE:
        return _CACHE[nsteps]
    nc = bacc.Bacc("TRN2", target_bir_lowering=False, debug=False,
                   num_devices=NCORES)
    d_x = nc.dram_tensor("xT", [nsteps, 128, KX, BC], BF16,
                         kind="ExternalInput").ap()
    d_w = nc.dram_tensor("W", [NK, 128, 4, H], BF16,
                         kind="ExternalInput").ap()
    d_ones = nc.dram_tensor("ones_row", [128, BC], BF16,
                            kind="ExternalInput").ap()
    d_eye = nc.dram_tensor("eye", [128, BC], BF16, kind="ExternalInput").ap()
    d_fcw = nc.dram_tensor("fcw", [BC, H], F32, kind="ExternalInput").ap()
    d_fcb = nc.dram_tensor("fcb", [BC, 1], F32, kind="ExternalInput").ap()
    d_out = nc.dram_tensor("out", [BC, 1], F32, kind="ExternalOutput").ap()

    with tile.TileContext(nc) as tc, ExitStack() as ctx:
        _body(tc, ctx, nsteps, d_x, d_w, d_ones, d_eye, d_fcw, d_fcb, d_out)
    nc.compile()
    _CACHE[nsteps] = nc
    return nc


def _body(tc, ctx, nsteps, d_x, d_w, d_ones, d_eye, d_fcw, d_fcb, d_out):
    nc = tc.nc
    const = ctx.enter_context(tc.tile_pool(name="const", bufs=1))
    xpool = ctx.enter_context(tc.tile_pool(name="x", bufs=4))
    gact = ctx.enter_context(tc.tile_pool(name="gact", bufs=2))
    state = ctx.enter_context(tc.tile_pool(name="state", bufs=2))
    psFI = ctx.enter_context(tc.tile_pool(name="psFI", bufs=2, space="PSUM"))
    psCO = ctx.enter_context(tc.tile_pool(name="psCO", bufs=2, space="PSUM"))
    psT = ctx.enter_context(tc.tile_pool(name="psT", bufs=1, space="PSUM"))

    # W layout in SBUF: [128, NK, 4, H]; moving slice for (k-chunk j,
    # gate g, hidden half hh) is sW[:, j, g, 256*hh : 256*hh+256].
    sW = const.tile([128, NK, 4, H], BF16)
    nc.sync.dma_start(out=sW[:], in_=d_w.rearrange("k p g n -> p k g n"))
    s_ones = const.tile([128, BC], BF16)
    nc.sync.dma_start(out=s_ones[:], in_=d_ones)
    s_eye = const.tile([128, BC], BF16)
    nc.sync.dma_start(out=s_eye[:], in_=d_eye)
    s_fcw = const.tile([BC, H], F32)
    nc.sync.dma_start(out=s_fcw[:], in_=d_fcw)
    s_fcb = const.tile([BC, 1], F32)
    nc.sync.dma_start(out=s_fcb[:], in_=d_fcb)

    c_prev = state.tile([128, HH], F32, tag="c")
    nc.vector.memset(c_prev[:], 0.0)

    def gate_mm(ps, half, ti, stat, k, start, stop):
        """One N=512 matmul into ps[64*half : 64*half+64, :].

        The host packs every W k-chunk so sW[:, k, 2*half+ti, :] is the
        contiguous [gate0-half | gate1-half] 512-wide moving slice for
        PSUM tile ti (0=FI, 1=CO) and column group `half`.  Each matmul
        covers the tile's full FD width, so there is exactly one PSUM
        accumulation chain per (tile, partition-half).
        """
        # skip_group_check: CoreSim's zero-region tracking is
        # partition-blind; the two column groups' chains target
        # disjoint partition halves of the same bank, which is safe
        # under the per-element has_written HW semantics (the staged
        # baseline used the same pattern and measured correct on HW).
        nc.tensor.matmul(ps[64 * half:64 * half + 64, :],
                         stat, sW[:, k, 2 * half + ti, :],
                         start=start, stop=stop, skip_group_check=True)

    def emit_x_and_bias(t, pFI, pCO):
        """x-part + bias matmuls for step t (h-independent).

        Column-group halves are emitted adjacently so the PE pairs them
        cleanly.
        """
        xs = xpool.tile([128, KX, BC], BF16, tag="xs")
        nc.sync.dma_start(out=xs[:], in_=d_x[t])
        for ti, ps in ((0, pFI), (1, pCO)):
            for kx in range(KX):
                for half in range(2):
                    gate_mm(ps, half, ti, xs[:, kx, :], KH + kx,
                            kx == 0, False)
            for half in range(2):
                gate_mm(ps, half, ti, s_ones[:], KH + KX,
                        False, t == 0)

    def emit_hpart(pFI, pCO, hTs):
        """h-part matmuls, FI tile first so sigmoid(f/i) starts early.

        hTs[idx] is the [128, 64] stationary for hidden chunk
        KORD[idx]; each has its own semaphore so chunk idx's matmuls
        start the moment its transpose+cast lands.  k-order k0,k2,k1,k3
        matches cast availability; column-group halves are adjacent for
        clean PE pairing.
        """
        for ti, ps in ((0, pFI), (1, pCO)):
            for idx in range(4):
                for half in range(2):
                    gate_mm(ps, half, ti, hTs[idx][:],
                            KORD[idx], False, idx == 3)

    # prologue: step 0 gates have no h contribution
    pFI = psFI.tile([128, 512], F32, tag="FI")
    pCO = psCO.tile([128, 512], F32, tag="CO")
    emit_x_and_bias(0, pFI, pCO)

    h = None
    for t in range(nsteps):
        last = t == nsteps - 1
        if not last:
            pFI_n = psFI.tile([128, 512], F32, tag="FI")
            pCO_n = psCO.tile([128, 512], F32, tag="CO")
            # high_priority keeps all 12 x/bias matmuls ahead of this
            # step's transposes in the PE's static order — otherwise the
            # scheduler interleaves them and the transposes' h-waits
            # head-of-line-block the later x matmuls
            with tc.high_priority():
                emit_x_and_bias(t + 1, pFI_n, pCO_n)

        # elementwise for step t, all [128, 256] (sigfi [128, 512])
        sigfi = gact.tile([128, 512], F32, tag="sigfi")
        nc.scalar.activation(sigfi[:], pFI[:], AF.Sigmoid)
        tcb = gact.tile([128, HH], F32, tag="tcb")
        nc.scalar.activation(tcb[:], pCO[:, 0:256], AF.Tanh)
        sigo = gact.tile([128, HH], BF16, tag="sigo")
        nc.scalar.activation(sigo[:], pCO[:, 256:512], AF.Sigmoid)

        u1 = gact.tile([128, HH], F32, tag="u1")
        nc.vector.tensor_mul(u1[:], c_prev[:], sigfi[:, 0:256])
        u2 = gact.tile([128, HH], F32, tag="u2")
        nc.vector.tensor_mul(u2[:], tcb[:], sigfi[:, 256:512])
        c_new = state.tile([128, HH], F32, tag="c")
        nc.vector.tensor_add(c_new[:], u1[:], u2[:])
        # tail pipelined in two FD-halves: tanh(c) -> h blocks (base-
        # aligned [64, 128] tiles, bf16) -> PE transposes -> hT casts.
        # FD-half a covers hidden chunks {0, 2}, half b covers {1, 3}.
        hblk = [None] * 4      # h blocks: [hla, hha, hlb, hhb]
        tch2 = [None, None]
        for cch in range(2):
            fd = slice(128 * cch, 128 * cch + 128)
            tch = gact.tile([128, 128], BF16, tag=f"tch{cch}")
            nc.scalar.activation(tch[:], c_new[:, fd], AF.Tanh)
            tch2[cch] = tch
            hl = state.tile([BC, 128], BF16, tag=f"hl{cch}")
            nc.vector.tensor_mul(hl[:], sigo[0:64, fd], tch[0:64, :])
            hh = state.tile([BC, 128], BF16, tag=f"hh{cch}")
            nc.vector.tensor_mul(hh[:], sigo[64:128, fd], tch[64:128, :])
            hblk[2 * cch] = hl
            hblk[2 * cch + 1] = hh
        c_prev = c_new

        if not last:
            # transpose h blocks -> hT for the next step's stationary.
            # One pT/hT tile pair per hidden chunk gives each its own
            # semaphore, so chunk idx's h-matmuls start the moment its
            # transpose + cast land.
            hTs = []
            for idx in range(4):
                pT = psT.tile([128, BC], BF16, tag=f"hTp{idx}")
                nc.tensor.transpose(pT[:], hblk[idx][:], s_eye[0:64, :])
                hT = state.tile([128, BC], BF16, tag=f"hT{idx}")
                nc.vector.tensor_copy(hT[:], pT[:])
                hTs.append(hT)
            emit_hpart(pFI_n, pCO_n, hTs)
            pFI, pCO = pFI_n, pCO_n

    # fc head: out = h @ fc_w.T + fc_b; h is in four [64, 128] blocks
    # (hid 0:128, 256:384, 128:256, 384:512) and s_fcw is [64, 512]
    # (fc_w broadcast over batch).
    m = gact.tile([BC, H], F32, tag="fcm")
    for bi, fd in ((0, 0), (1, 256), (2, 128), (3, 384)):
        nc.vector.tensor_mul(m[:, fd:fd + 128], hblk[bi][:],
                             s_fcw[:, fd:fd + 128])
    r = gact.tile([BC, 1], F32, tag="fcr")
    nc.vector.tensor_reduce(r[:], m[:], axis=mybir.AxisListType.X,
                            op=mybir.AluOpType.add)
    ro = gact.tile([BC, 1], F32, tag="fco")
    nc.vector.tensor_add(ro[:], r[:], s_fcb[:])
    nc.sync.dma_start(out=d_out, in_=ro[:])


def _prep_core_inputs(x, W_w, W_b, fc_w, fc_b, core, nsteps=T):
    """Host-side shard + relayout for one core."""
    xs = x[core * BC:(core + 1) * BC, :nsteps]          # [BC, t, I]
    xt = np.ascontiguousarray(xs.transpose(1, 2, 0))    # [t, I, BC]
    xt = xt.reshape(nsteps, KX, 128, BC).transpose(0, 2, 1, 3)
    xt = np.ascontiguousarray(xt)                       # [t, 128, KX, BC]

    # W layout: [NK, 128, 4, H]; k-chunks 0..3 = Wh.T, 4..5 = Wx.T,
    # 6 = bias row.  Every k-chunk is packed in (tile, half) quadrants
    # q = 2*half + ti along dim 2:
    #   q=0: [f-lo | i-lo]   q=1: [chat-lo | o-lo]
    #   q=2: [f-hi | i-hi]   q=3: [chat-hi | o-hi]
    wfull = W_w.T.reshape(H + I, 4, 2, HH)          # [768, gate, half, 256]
    wb = W_b.reshape(4, 2, HH)                      # [gate, half, 256]
    wt = np.zeros((NK * 128, 4, H), dtype=np.float32)
    for half in range(2):
        for ti, gpair in enumerate((G_FI, G_CO)):
            q = 2 * half + ti
            wt[:H + I, q, 0:HH] = wfull[:, gpair[0], half]
            wt[:H + I, q, HH:H] = wfull[:, gpair[1], half]
            wt[H + I, q, 0:HH] = wb[gpair[0], half]
            wt[H + I, q, HH:H] = wb[gpair[1], half]
    wt = np.ascontiguousarray(wt.reshape(NK, 128, 4, H))

    ones_row = np.zeros((128, BC), dtype=np.float32)
    ones_row[0, :] = 1.0
    eye = np.concatenate([np.eye(BC, dtype=np.float32)] * 2, axis=0)
    # fc_w broadcast over the 64 batch partitions: [64, 512]
    fcw = np.ascontiguousarray(np.broadcast_to(fc_w.reshape(1, H), (BC, H)))
    fcb = np.full((BC, 1), np.float32(fc_b[0]), dtype=np.float32)

    import ml_dtypes
    bf = ml_dtypes.bfloat16
    return {"xT": xt.astype(bf), "W": wt.astype(bf),
            "ones_row": ones_row.astype(bf), "eye": eye.astype(bf),
            "fcw": fcw, "fcb": fcb}


def kernel(x, W_w, W_b, fc_w, fc_b):
    x = np.asarray(x, dtype=np.float32)
    W_w = np.asarray(W_w, dtype=np.float32)
    W_b = np.asarray(W_b, dtype=np.float32)
    fc_w = np.asarray(fc_w, dtype=np.float32)
    fc_b = np.asarray(fc_b, dtype=np.float32)

    nc = _build(T)
    in_maps = [_prep_core_inputs(x, W_w, W_b, fc_w, fc_b, c)
               for c in range(NCORES)]
    res = run_bass_kernel_spmd(nc, in_maps, list(range(NCORES))).results
    return np.concatenate([res[c]["out"] for c in range(NCORES)], axis=0)


# revision 38
# speedup vs baseline: 1.3592x; 1.3592x over previous
"""Trainium2 Bass kernel for CustomLSTMForecast.

B=512, T=256, I=256, H=512. Data-parallel: batch sharded 8 ways (64
rows/core), LSTM + fc weights replicated.  Measured ~1.37 ms on HW
(vs 1.87 ms for the [64, 512]-layout baseline).

Per-core design (batch m = 64):
  Gate layout: gates land in PSUM with each gate as [128, 256] --
  partitions 0:64 = batch x hidden-lo (0:256), partitions 64:128 =
  batch x hidden-hi (256:512).  Two PSUM tiles per step, FD-packed so
  one sigmoid covers f and i:
     pFI [128, 512]: FD 0:256 = f-gate, 256:512 = i-gate
     pCO [128, 512]: FD 0:256 = c_hat,  256:512 = o-gate
  Every matmul is N=512 over a tile's full FD width (one PSUM
  accumulation chain per tile/partition-half); the host packs W into
  (tile, column-half) quadrants to make those moving slices
  contiguous.  The two column groups (out base partitions 0/64) run
  concurrently on the PE.  K-chunks: 4x h, 2x x, 1x ones row (bias).

  All elementwise runs at [128, 256]/[128, 512] (full partition
  occupancy -- FD-bound ACT/DVE cost is nearly halved vs a [64, 512]
  layout).  The h path is bf16 (2x DVE mode); the c path stays f32.

  The serial chain per step is h-matmuls(FI) -> sigmoid(fi) ->
  [tanh(c_hat) -> u2 -> c] -> tanh(c) -> h -> PE transpose -> cast ->
  next h-matmuls.  To shorten it: tanh(c)/h/transpose/cast are
  pipelined in two FD-halves (hidden chunks {0,2} then {1,3}, giving
  the KORD h-matmul order), each hidden chunk's transpose/cast has its
  own tiles and semaphores, x-part matmuls of step t+1 fill the PE
  during the elementwise of step t, and column-group pairs are emitted
  adjacently so the PE co-issues them.

  PE transposes require base-partition-0 operands (base-64 stationary
  inputs hang the exec unit), hence the base-aligned [64, 128] h
  blocks.
"""
from contextlib import ExitStack

import numpy as np

import concourse.bass as bass
import concourse.tile as tile
from concourse import bacc, mybir
from concourse.bass_utils import run_bass_kernel_spmd

F32 = mybir.dt.float32
BF16 = mybir.dt.bfloat16
AF = mybir.ActivationFunctionType

B, T, I, H = 512, 256, 256, 512
NCORES = 8
BC = B // NCORES          # 64 batch rows per core
HH = H // 2               # 256: free size of a [128, 256] gate tile
KH = H // 128             # 4 hidden k-chunks
KX = I // 128             # 2 input k-chunks
NK = KH + KX + 1          # 7 k-chunks incl bias row

# gate chunk order in W_w: f=0, i=1, o=2, chat=3
# pFI holds (f, i) in FD halves; pCO holds (chat, o)
G_FI = (0, 1)
G_CO = (3, 2)
# h-part k-chunk emission order: FD-half a of the elementwise tail
# yields hidden chunks 0 and 2, half b yields 1 and 3
KORD = (0, 2, 1, 3)

_CACHE = {}


def _build(nsteps=T):
    if nsteps in _CACHE:
        return _CACHE[nsteps]
    nc = bacc.Bacc("TRN2", target_bir_lowering=False, debug=False,
                   num_devices=NCORES)
    d_x = nc.dram_tensor("xT", [nsteps, 128, KX, BC], BF16,
                         kind="ExternalInput").ap()
    d_w = nc.dram_tensor("W", [NK, 128, 4, H], BF16,
                         kind="ExternalInput").ap()
    d_ones = nc.dram_tensor("ones_row", [128, BC], BF16,
                            kind="ExternalInput").ap()
    d_eye = nc.dram_tensor("eye", [128, BC], BF16, kind="ExternalInput").ap()
    d_fcw = nc.dram_tensor("fcw", [BC, H], F32, kind="ExternalInput").ap()
    d_fcb = nc.dram_tensor("fcb", [BC, 1], F32, kind="ExternalInput").ap()
    d_out = nc.dram_tensor("out", [BC, 1], F32, kind="ExternalOutput").ap()

    with tile.TileContext(nc) as tc, ExitStack() as ctx:
        _body(tc, ctx, nsteps, d_x, d_w, d_ones, d_eye, d_fcw, d_fcb, d_out)
    nc.compile()
    _CACHE[nsteps] = nc
    return nc


def _body(tc, ctx, nsteps, d_x, d_w, d_ones, d_eye, d_fcw, d_fcb, d_out):
    nc = tc.nc
    const = ctx.enter_context(tc.tile_pool(name="const", bufs=1))
    xpool = ctx.enter_context(tc.tile_pool(name="x", bufs=4))
    gact = ctx.enter_context(tc.tile_pool(name="gact", bufs=2))
    state = ctx.enter_context(tc.tile_pool(name="state", bufs=2))
    psFI = ctx.enter_context(tc.tile_pool(name="psFI", bufs=2, space="PSUM"))
    psCO = ctx.enter_context(tc.tile_pool(name="psCO", bufs=2, space="PSUM"))
    psT = ctx.enter_context(tc.tile_pool(name="psT", bufs=1, space="PSUM"))

    # W layout in SBUF: [128, NK, 4, H]; moving slice for (k-chunk j,
    # gate g, hidden half hh) is sW[:, j, g, 256*hh : 256*hh+256].
    sW = const.tile([128, NK, 4, H], BF16)
    nc.sync.dma_start(out=sW[:], in_=d_w.rearrange("k p g n -> p k g n"))
    s_ones = const.tile([128, BC], BF16)
    nc.sync.dma_start(out=s_ones[:], in_=d_ones)
    s_eye = const.tile([128, BC], BF16)
    nc.sync.dma_start(out=s_eye[:], in_=d_eye)
    s_fcw = const.tile([BC, H], F32)
    nc.sync.dma_start(out=s_fcw[:], in_=d_fcw)
    s_fcb = const.tile([BC, 1], F32)
    nc.sync.dma_start(out=s_fcb[:], in_=d_fcb)

    # cell state kept as two FD-half tiles so the u2/c/tanh chain can
    # pipeline per half
    c_a0 = state.tile([128, 128], F32, tag="ca")
    c_b0 = state.tile([128, 128], F32, tag="cb")
    c_prev = [c_a0, c_b0]
    nc.vector.memset(c_a0[:], 0.0)
    nc.vector.memset(c_b0[:], 0.0)

    def gate_mm(ps, half, ti, stat, k, start, stop):
        """One N=512 matmul into ps[64*half : 64*half+64, :].

        The host packs every W k-chunk so sW[:, k, 2*half+ti, :] is the
        contiguous [gate0-half | gate1-half] 512-wide moving slice for
        PSUM tile ti (0=FI, 1=CO) and column group `half`.  Each matmul
        covers the tile's full FD width, so there is exactly one PSUM
        accumulation chain per (tile, partition-half).
        """
        # skip_group_check: CoreSim's zero-region tracking is
        # partition-blind; the two column groups' chains target
        # disjoint partition halves of the same bank, which is safe
        # under the per-element has_written HW semantics (the staged
        # baseline used the same pattern and measured correct on HW).
        nc.tensor.matmul(ps[64 * half:64 * half + 64, :],
                         stat, sW[:, k, 2 * half + ti, :],
                         start=start, stop=stop, skip_group_check=True)

    def emit_x_and_bias(t, pFI, pCO):
        """x-part + bias matmuls for step t (h-independent).

        Column-group halves are emitted adjacently so the PE pairs them
        cleanly.
        """
        xs = xpool.tile([128, KX, BC], BF16, tag="xs")
        nc.sync.dma_start(out=xs[:], in_=d_x[t])
        for ti, ps in ((0, pFI), (1, pCO)):
            for kx in range(KX):
                for half in range(2):
                    gate_mm(ps, half, ti, xs[:, kx, :], KH + kx,
                            kx == 0, False)
            for half in range(2):
                gate_mm(ps, half, ti, s_ones[:], KH + KX,
                        False, t == 0)

    def emit_hpart(pFI, pCO, hTs):
        """h-part matmuls, FI tile first so sigmoid(f/i) starts early.

        hTs[idx] is the [128, 64] stationary for hidden chunk
        KORD[idx]; each has its own semaphore so chunk idx's matmuls
        start the moment its transpose+cast lands.  k-order k0,k2,k1,k3
        matches cast availability; column-group halves are adjacent for
        clean PE pairing.
        """
        for ti, ps in ((0, pFI), (1, pCO)):
            for idx in range(4):
                for half in range(2):
                    gate_mm(ps, half, ti, hTs[idx][:],
                            KORD[idx], False, idx == 3)

    # prologue: step 0 gates have no h contribution
    pFI = psFI.tile([128, 512], F32, tag="FI")
    pCO = psCO.tile([128, 512], F32, tag="CO")
    emit_x_and_bias(0, pFI, pCO)

    h = None
    for t in range(nsteps):
        last = t == nsteps - 1
        if not last:
            pFI_n = psFI.tile([128, 512], F32, tag="FI")
            pCO_n = psCO.tile([128, 512], F32, tag="CO")
            emit_x_and_bias(t + 1, pFI_n, pCO_n)

        # elementwise for step t, all [128, 256] (sigfi [128, 512]).
        # u1 is computed per FD-half early (off the critical chain);
        # u2 -> c runs per FD-half so tanh(c) half a starts ~400ns
        # sooner than a full-width u2 -> c would allow.
        sigfi = gact.tile([128, 512], F32, tag="sigfi")
        nc.scalar.activation(sigfi[:], pFI[:], AF.Sigmoid)
        tcb = gact.tile([128, HH], F32, tag="tcb")
        nc.scalar.activation(tcb[:], pCO[:, 0:256], AF.Tanh)
        sigo = gact.tile([128, HH], BF16, tag="sigo")
        nc.scalar.activation(sigo[:], pCO[:, 256:512], AF.Sigmoid)

        u1h = []
        for cch in range(2):
            fd = slice(128 * cch, 128 * cch + 128)
            u1 = gact.tile([128, 128], F32, tag=f"u1{cch}")
            nc.vector.tensor_mul(u1[:], c_prev[cch][:], sigfi[:, fd])
            u1h.append(u1)
        c_new = []
        for cch in range(2):
            fd = slice(128 * cch, 128 * cch + 128)
            u2 = gact.tile([128, 128], F32, tag=f"u2{cch}")
            nc.vector.tensor_mul(u2[:], tcb[:, fd],
                                 sigfi[:, 256 + 128 * cch:384 + 128 * cch])
            cc = state.tile([128, 128], F32, tag=f"c{'ab'[cch]}")
            nc.vector.tensor_add(cc[:], u1h[cch][:], u2[:])
            c_new.append(cc)
        # tail pipelined in two FD-halves: tanh(c) -> h blocks (base-
        # aligned [64, 128] tiles, bf16) -> PE transposes -> hT casts.
        # FD-half a covers hidden chunks {0, 2}, half b covers {1, 3}.
        hblk = [None] * 4      # h blocks: [hla, hha, hlb, hhb]
        tch2 = [None, None]
        for cch in range(2):
            fd = slice(128 * cch, 128 * cch + 128)
            tch = gact.tile([128, 128], BF16, tag=f"tch{cch}")
            nc.scalar.activation(tch[:], c_new[cch][:], AF.Tanh)
            tch2[cch] = tch
            hl = state.tile([BC, 128], BF16, tag=f"hl{cch}")
            nc.vector.tensor_mul(hl[:], sigo[0:64, fd], tch[0:64, :])
            hh = state.tile([BC, 128], BF16, tag=f"hh{cch}")
            nc.vector.tensor_mul(hh[:], sigo[64:128, fd], tch[64:128, :])
            hblk[2 * cch] = hl
            hblk[2 * cch + 1] = hh
        c_prev = c_new

        if not last:
            # transpose h blocks -> hT for the next step's stationary.
            # One pT/hT tile pair per hidden chunk gives each its own
            # semaphore, so chunk idx's h-matmuls start the moment its
            # transpose + cast land.
            hTs = []
            for idx in range(4):
                pT = psT.tile([128, BC], BF16, tag=f"hTp{idx}")
                nc.tensor.transpose(pT[:], hblk[idx][:], s_eye[0:64, :])
                hT = state.tile([128, BC], BF16, tag=f"hT{idx}")
                nc.vector.tensor_copy(hT[:], pT[:])
                hTs.append(hT)
            emit_hpart(pFI_n, pCO_n, hTs)
            pFI, pCO = pFI_n, pCO_n

    # fc head: out = h @ fc_w.T + fc_b; h is in four [64, 128] blocks
    # (hid 0:128, 256:384, 128:256, 384:512) and s_fcw is [64, 512]
    # (fc_w broadcast over batch).
    m = gact.tile([BC, H], F32, tag="fcm")
    for bi, fd in ((0, 0), (1, 256), (2, 128), (3, 384)):
        nc.vector.tensor_mul(m[:, fd:fd + 128], hblk[bi][:],
                             s_fcw[:, fd:fd + 128])
    r = gact.tile([BC, 1], F32, tag="fcr")
    nc.vector.tensor_reduce(r[:], m[:], axis=mybir.AxisListType.X,
                            op=mybir.AluOpType.add)
    ro = gact.tile([BC, 1], F32, tag="fco")
    nc.vector.tensor_add(ro[:], r[:], s_fcb[:])
    nc.sync.dma_start(out=d_out, in_=ro[:])


def _prep_core_inputs(x, W_w, W_b, fc_w, fc_b, core, nsteps=T):
    """Host-side shard + relayout for one core."""
    xs = x[core * BC:(core + 1) * BC, :nsteps]          # [BC, t, I]
    xt = np.ascontiguousarray(xs.transpose(1, 2, 0))    # [t, I, BC]
    xt = xt.reshape(nsteps, KX, 128, BC).transpose(0, 2, 1, 3)
    xt = np.ascontiguousarray(xt)                       # [t, 128, KX, BC]

    # W layout: [NK, 128, 4, H]; k-chunks 0..3 = Wh.T, 4..5 = Wx.T,
    # 6 = bias row.  Every k-chunk is packed in (tile, half) quadrants
    # q = 2*half + ti along dim 2:
    #   q=0: [f-lo | i-lo]   q=1: [chat-lo | o-lo]
    #   q=2: [f-hi | i-hi]   q=3: [chat-hi | o-hi]
    wfull = W_w.T.reshape(H + I, 4, 2, HH)          # [768, gate, half, 256]
    wb = W_b.reshape(4, 2, HH)                      # [gate, half, 256]
    wt = np.zeros((NK * 128, 4, H), dtype=np.float32)
    for half in range(2):
        for ti, gpair in enumerate((G_FI, G_CO)):
            q = 2 * half + ti
            wt[:H + I, q, 0:HH] = wfull[:, gpair[0], half]
            wt[:H + I, q, HH:H] = wfull[:, gpair[1], half]
            wt[H + I, q, 0:HH] = wb[gpair[0], half]
            wt[H + I, q, HH:H] = wb[gpair[1], half]
    wt = np.ascontiguousarray(wt.reshape(NK, 128, 4, H))

    ones_row = np.zeros((128, BC), dtype=np.float32)
    ones_row[0, :] = 1.0
    eye = np.concatenate([np.eye(BC, dtype=np.float32)] * 2, axis=0)
    # fc_w broadcast over the 64 batch partitions: [64, 512]
    fcw = np.ascontiguousarray(np.broadcast_to(fc_w.reshape(1, H), (BC, H)))
    fcb = np.full((BC, 1), np.float32(fc_b[0]), dtype=np.float32)

    import ml_dtypes
    bf = ml_dtypes.bfloat16
    return {"xT": xt.astype(bf), "W": wt.astype(bf),
            "ones_row": ones_row.astype(bf), "eye": eye.astype(bf),
            "fcw": fcw, "fcb": fcb}


def kernel(x, W_w, W_b, fc_w, fc_b):
    x = np.asarray(x, dtype=np.float32)
    W_w = np.asarray(W_w, dtype=np.float32)
    W_b = np.asarray(W_b, dtype=np.float32)
    fc_w = np.asarray(fc_w, dtype=np.float32)
    fc_b = np.asarray(fc_b, dtype=np.float32)

    nc = _build(T)
    in_maps = [_prep_core_inputs(x, W_w, W_b, fc_w, fc_b, c)
               for c in range(NCORES)]
    res = run_bass_kernel_spmd(nc, in_maps, list(range(NCORES))).results
    return np.concatenate([res[c]["out"] for c in range(NCORES)], axis=0)


# revision 40
# speedup vs baseline: 1.3798x; 1.0152x over previous
"""Trainium2 Bass kernel for CustomLSTMForecast.

B=512, T=256, I=256, H=512. Data-parallel: batch sharded 8 ways (64
rows/core), LSTM + fc weights replicated.  Measured 1.310 ms on HW
(vs 1.87 ms for the [64, 512]-layout baseline).

Per-core design (batch m = 64):
  Gate layout: gates land in PSUM with each gate as [128, 256] --
  partitions 0:64 = batch x hidden-lo (0:256), partitions 64:128 =
  batch x hidden-hi (256:512).  Two PSUM tiles per step, FD-packed so
  one sigmoid covers f and i:
     pFI [128, 512]: FD 0:256 = f-gate, 256:512 = i-gate
     pCO [128, 512]: FD 0:256 = c_hat,  256:512 = o-gate
  Every matmul is N=512 over a tile's full FD width (one PSUM
  accumulation chain per tile/partition-half); the host packs W into
  (tile, column-half) quadrants to make those moving slices
  contiguous.  The two column groups (out base partitions 0/64) run
  concurrently on the PE.  K-chunks: 4x h, 2x x, 1x ones row (bias).

  All elementwise runs at [128, 256]/[128, 512] (full partition
  occupancy -- FD-bound ACT/DVE cost is nearly halved vs a [64, 512]
  layout).  The h path is bf16 (2x DVE mode); the c path stays f32.

  The serial chain per step is h-matmuls(FI) -> sigmoid(fi) ->
  [tanh(c_hat) -> u2 -> c] -> tanh(c) -> h -> PE transpose -> cast ->
  next h-matmuls.  To shorten it: u2/c and tanh(c)/h/transpose/cast
  are pipelined in two FD-halves (hidden chunks {0,2} then {1,3},
  giving the KORD h-matmul order; c lives as two [128, 128] tiles),
  each hidden chunk's transpose/cast has its own tiles and semaphores,
  x-part matmuls of step t+1 fill the PE during the elementwise of
  step t, and column-group pairs are emitted adjacently so the PE
  co-issues them.

  PE transposes require base-partition-0 operands (base-64 stationary
  inputs hang the exec unit), hence the base-aligned [64, 128] h
  blocks.
"""
from contextlib import ExitStack

import numpy as np

import concourse.bass as bass
import concourse.tile as tile
from concourse import bacc, mybir
from concourse.bass_utils import run_bass_kernel_spmd

F32 = mybir.dt.float32
BF16 = mybir.dt.bfloat16
AF = mybir.ActivationFunctionType

B, T, I, H = 512, 256, 256, 512
NCORES = 8
BC = B // NCORES          # 64 batch rows per core
HH = H // 2               # 256: free size of a [128, 256] gate tile
KH = H // 128             # 4 hidden k-chunks
KX = I // 128             # 2 input k-chunks
NK = KH + KX + 1          # 7 k-chunks incl bias row

# gate chunk order in W_w: f=0, i=1, o=2, chat=3
# pFI holds (f, i) in FD halves; pCO holds (chat, o)
G_FI = (0, 1)
G_CO = (3, 2)
# h-part k-chunk emission order: FD-half a of the elementwise tail
# yields hidden chunks 0 and 2, half b yields 1 and 3
KORD = (0, 2, 1, 3)

_CACHE = {}


def _build(nsteps=T):
    if nsteps in _CACHE:
        return _CACHE[nsteps]
    nc = bacc.Bacc("TRN2", target_bir_lowering=False, debug=False,
                   num_devices=NCORES)
    d_x = nc.dram_tensor("xT", [nsteps, 128, KX, BC], BF16,
                         kind="ExternalInput").ap()
    d_w = nc.dram_tensor("W", [NK, 128, 4, H], BF16,
                         kind="ExternalInput").ap()
    d_ones = nc.dram_tensor("ones_row", [128, BC], BF16,
                            kind="ExternalInput").ap()
    d_eye = nc.dram_tensor("eye", [128, BC], BF16, kind="ExternalInput").ap()
    d_fcw = nc.dram_tensor("fcw", [BC, H], F32, kind="ExternalInput").ap()
    d_fcb = nc.dram_tensor("fcb", [BC, 1], F32, kind="ExternalInput").ap()
    d_out = nc.dram_tensor("out", [BC, 1], F32, kind="ExternalOutput").ap()

    with tile.TileContext(nc) as tc, ExitStack() as ctx:
        _body(tc, ctx, nsteps, d_x, d_w, d_ones, d_eye, d_fcw, d_fcb, d_out)
    nc.compile()
    _CACHE[nsteps] = nc
    return nc


def _body(tc, ctx, nsteps, d_x, d_w, d_ones, d_eye, d_fcw, d_fcb, d_out):
    nc = tc.nc
    const = ctx.enter_context(tc.tile_pool(name="const", bufs=1))
    xpool = ctx.enter_context(tc.tile_pool(name="x", bufs=4))
    gact = ctx.enter_context(tc.tile_pool(name="gact", bufs=2))
    state = ctx.enter_context(tc.tile_pool(name="state", bufs=2))
    psFI = ctx.enter_context(tc.tile_pool(name="psFI", bufs=2, space="PSUM"))
    psCO = ctx.enter_context(tc.tile_pool(name="psCO", bufs=2, space="PSUM"))
    psT = ctx.enter_context(tc.tile_pool(name="psT", bufs=1, space="PSUM"))

    # W layout in SBUF: [128, NK, 4, H]; moving slice for (k-chunk j,
    # gate g, hidden half hh) is sW[:, j, g, 256*hh : 256*hh+256].
    sW = const.tile([128, NK, 4, H], BF16)
    nc.sync.dma_start(out=sW[:], in_=d_w.rearrange("k p g n -> p k g n"))
    s_ones = const.tile([128, BC], BF16)
    nc.sync.dma_start(out=s_ones[:], in_=d_ones)
    s_eye = const.tile([128, BC], BF16)
    nc.sync.dma_start(out=s_eye[:], in_=d_eye)
    s_fcw = const.tile([BC, H], F32)
    nc.sync.dma_start(out=s_fcw[:], in_=d_fcw)
    s_fcb = const.tile([BC, 1], F32)
    nc.sync.dma_start(out=s_fcb[:], in_=d_fcb)

    # cell state kept as two FD-half tiles so the u2/c/tanh chain can
    # pipeline per half
    c_a0 = state.tile([128, 128], F32, tag="ca")
    c_b0 = state.tile([128, 128], F32, tag="cb")
    c_prev = [c_a0, c_b0]
    nc.vector.memset(c_a0[:], 0.0)
    nc.vector.memset(c_b0[:], 0.0)

    def gate_mm(ps, half, ti, stat, k, start, stop):
        """One N=512 matmul into ps[64*half : 64*half+64, :].

        The host packs every W k-chunk so sW[:, k, 2*half+ti, :] is the
        contiguous [gate0-half | gate1-half] 512-wide moving slice for
        PSUM tile ti (0=FI, 1=CO) and column group `half`.  Each matmul
        covers the tile's full FD width, so there is exactly one PSUM
        accumulation chain per (tile, partition-half).
        """
        # skip_group_check: CoreSim's zero-region tracking is
        # partition-blind; the two column groups' chains target
        # disjoint partition halves of the same bank, which is safe
        # under the per-element has_written HW semantics (the staged
        # baseline used the same pattern and measured correct on HW).
        nc.tensor.matmul(ps[64 * half:64 * half + 64, :],
                         stat, sW[:, k, 2 * half + ti, :],
                         start=start, stop=stop, skip_group_check=True)

    def emit_x_and_bias(t, pFI, pCO):
        """x-part + bias matmuls for step t (h-independent).

        Column-group halves are emitted adjacently so the PE pairs them
        cleanly.
        """
        xs = xpool.tile([128, KX, BC], BF16, tag="xs")
        nc.sync.dma_start(out=xs[:], in_=d_x[t])
        for ti, ps in ((0, pFI), (1, pCO)):
            for kx in range(KX):
                for half in range(2):
                    gate_mm(ps, half, ti, xs[:, kx, :], KH + kx,
                            kx == 0, False)
            for half in range(2):
                gate_mm(ps, half, ti, s_ones[:], KH + KX,
                        False, t == 0)

    def emit_hpart(pFI, pCO, hTs):
        """h-part matmuls, FI tile first so sigmoid(f/i) starts early.

        hTs[idx] is the [128, 64] stationary for hidden chunk
        KORD[idx]; each has its own semaphore so chunk idx's matmuls
        start the moment its transpose+cast lands.  k-order k0,k2,k1,k3
        matches cast availability; column-group halves are adjacent for
        clean PE pairing.
        """
        for ti, ps in ((0, pFI), (1, pCO)):
            for idx in range(4):
                for half in range(2):
                    gate_mm(ps, half, ti, hTs[idx][:],
                            KORD[idx], False, idx == 3)

    # prologue: step 0 gates have no h contribution
    pFI = psFI.tile([128, 512], F32, tag="FI")
    pCO = psCO.tile([128, 512], F32, tag="CO")
    emit_x_and_bias(0, pFI, pCO)

    h = None
    for t in range(nsteps):
        last = t == nsteps - 1
        if not last:
            pFI_n = psFI.tile([128, 512], F32, tag="FI")
            pCO_n = psCO.tile([128, 512], F32, tag="CO")
            emit_x_and_bias(t + 1, pFI_n, pCO_n)

        # elementwise for step t, all [128, 256] (sigfi [128, 512]).
        # u1 is computed per FD-half early (off the critical chain);
        # u2 -> c runs per FD-half so tanh(c) half a starts ~400ns
        # sooner than a full-width u2 -> c would allow.
        sigfi = gact.tile([128, 512], F32, tag="sigfi")
        nc.scalar.activation(sigfi[:], pFI[:], AF.Sigmoid)
        tcb = gact.tile([128, HH], F32, tag="tcb")
        nc.scalar.activation(tcb[:], pCO[:, 0:256], AF.Tanh)
        sigo = gact.tile([128, HH], BF16, tag="sigo")
        nc.scalar.activation(sigo[:], pCO[:, 256:512], AF.Sigmoid)

        u1h = []
        for cch in range(2):
            fd = slice(128 * cch, 128 * cch + 128)
            u1 = gact.tile([128, 128], F32, tag=f"u1{cch}")
            nc.vector.tensor_mul(u1[:], c_prev[cch][:], sigfi[:, fd])
            u1h.append(u1)
        c_new = []
        for cch in range(2):
            fd = slice(128 * cch, 128 * cch + 128)
            u2 = gact.tile([128, 128], F32, tag=f"u2{cch}")
            nc.vector.tensor_mul(u2[:], tcb[:, fd],
                                 sigfi[:, 256 + 128 * cch:384 + 128 * cch])
            cc = state.tile([128, 128], F32, tag=f"c{'ab'[cch]}")
            nc.vector.tensor_add(cc[:], u1h[cch][:], u2[:])
            c_new.append(cc)
        # tail pipelined in two FD-halves: tanh(c) -> h blocks (base-
        # aligned [64, 128] tiles, bf16) -> PE transposes -> hT casts.
        # FD-half a covers hidden chunks {0, 2}, half b covers {1, 3}.
        hblk = [None] * 4      # h blocks: [hla, hha, hlb, hhb]
        tch2 = [None, None]
        for cch in range(2):
            fd = slice(128 * cch, 128 * cch + 128)
            tch = gact.tile([128, 128], BF16, tag=f"tch{cch}")
            nc.scalar.activation(tch[:], c_new[cch][:], AF.Tanh)
            tch2[cch] = tch
            hl = state.tile([BC, 128], BF16, tag=f"hl{cch}")
            nc.vector.tensor_mul(hl[:], sigo[0:64, fd], tch[0:64, :])
            hh = state.tile([BC, 128], BF16, tag=f"hh{cch}")
            nc.vector.tensor_mul(hh[:], sigo[64:128, fd], tch[64:128, :])
            hblk[2 * cch] = hl
            hblk[2 * cch + 1] = hh
        c_prev = c_new

        if not last:
            # transpose h blocks -> hT for the next step's stationary.
            # One pT/hT tile pair per hidden chunk gives each its own
            # semaphore, so chunk idx's h-matmuls start the moment its
            # transpose + cast land.
            hTs = []
            for idx in range(4):
                pT = psT.tile([128, BC], BF16, tag=f"hTp{idx}")
                nc.tensor.transpose(pT[:], hblk[idx][:], s_eye[0:64, :])
                hT = state.tile([128, BC], BF16, tag=f"hT{idx}")
                # cast on the Scalar engine: ACT is idle between
                # tanh(c) and the next sigmoid, while the DVE is busy
                # with the b-half h-muls
                nc.scalar.copy(hT[:], pT[:])
                hTs.append(hT)
            emit_hpart(pFI_n, pCO_n, hTs)
            pFI, pCO = pFI_n, pCO_n

    # fc head: out = h @ fc_w.T + fc_b; h is in four [64, 128] blocks
    # (hid 0:128, 256:384, 128:256, 384:512) and s_fcw is [64, 512]
    # (fc_w broadcast over batch).
    m = gact.tile([BC, H], F32, tag="fcm")
    for bi, fd in ((0, 0), (1, 256), (2, 128), (3, 384)):
        nc.vector.tensor_mul(m[:, fd:fd + 128], hblk[bi][:],
                             s_fcw[:, fd:fd + 128])
    r = gact.tile([BC, 1], F32, tag="fcr")
    nc.vector.tensor_reduce(r[:], m[:], axis=mybir.AxisListType.X,
                            op=mybir.AluOpType.add)
    ro = gact.tile([BC, 1], F32, tag="fco")
    nc.vector.tensor_add(ro[:], r[:], s_fcb[:])
    nc.sync.dma_start(out=d_out, in_=ro[:])


def _prep_core_inputs(x, W_w, W_b, fc_w, fc_b, core, nsteps=T):
    """Host-side shard + relayout for one core."""
    xs = x[core * BC:(core + 1) * BC, :nsteps]          # [BC, t, I]
    xt = np.ascontiguousarray(xs.transpose(1, 2, 0))    # [t, I, BC]
    xt = xt.reshape(nsteps, KX, 128, BC).transpose(0, 2, 1, 3)
    xt = np.ascontiguousarray(xt)                       # [t, 128, KX, BC]

    # W layout: [NK, 128, 4, H]; k-chunks 0..3 = Wh.T, 4..5 = Wx.T,
    # 6 = bias row.  Every k-chunk is packed in (tile, half) quadrants
    # q = 2*half + ti along dim 2:
    #   q=0: [f-lo | i-lo]   q=1: [chat-lo | o-lo]
    #   q=2: [f-hi | i-hi]   q=3: [chat-hi | o-hi]
    wfull = W_w.T.reshape(H + I, 4, 2, HH)          # [768, gate, half, 256]
    wb = W_b.reshape(4, 2, HH)                      # [gate, half, 256]
    wt = np.zeros((NK * 128, 4, H), dtype=np.float32)
    for half in range(2):
        for ti, gpair in enumerate((G_FI, G_CO)):
            q = 2 * half + ti
            wt[:H + I, q, 0:HH] = wfull[:, gpair[0], half]
            wt[:H + I, q, HH:H] = wfull[:, gpair[1], half]
            wt[H + I, q, 0:HH] = wb[gpair[0], half]
            wt[H + I, q, HH:H] = wb[gpair[1], half]
    wt = np.ascontiguousarray(wt.reshape(NK, 128, 4, H))

    ones_row = np.zeros((128, BC), dtype=np.float32)
    ones_row[0, :] = 1.0
    eye = np.concatenate([np.eye(BC, dtype=np.float32)] * 2, axis=0)
    # fc_w broadcast over the 64 batch partitions: [64, 512]
    fcw = np.ascontiguousarray(np.broadcast_to(fc_w.reshape(1, H), (BC, H)))
    fcb = np.full((BC, 1), np.float32(fc_b[0]), dtype=np.float32)

    import ml_dtypes
    bf = ml_dtypes.bfloat16
    return {"xT": xt.astype(bf), "W": wt.astype(bf),
            "ones_row": ones_row.astype(bf), "eye": eye.astype(bf),
            "fcw": fcw, "fcb": fcb}


def kernel(x, W_w, W_b, fc_w, fc_b):
    x = np.asarray(x, dtype=np.float32)
    W_w = np.asarray(W_w, dtype=np.float32)
    W_b = np.asarray(W_b, dtype=np.float32)
    fc_w = np.asarray(fc_w, dtype=np.float32)
    fc_b = np.asarray(fc_b, dtype=np.float32)

    nc = _build(T)
    in_maps = [_prep_core_inputs(x, W_w, W_b, fc_w, fc_b, c)
               for c in range(NCORES)]
    res = run_bass_kernel_spmd(nc, in_maps, list(range(NCORES))).results
    return np.concatenate([res[c]["out"] for c in range(NCORES)], axis=0)
